# revision 17
# baseline (speedup 1.0000x reference)
"""Trilinear scatter-add (splat) + Huber loss kernel for Trainium2, 8 NeuronCores.

Strategy
--------
reference computes:  huber_sum(splat(coords+pred) - splat(coords+gt)) over a
128^3 grid with trilinear weights and vals=1.

Key identity: the trilinear corner weights of a point with pixel coordinate p
along one axis are exactly  hat_j(p) = relu(1 - |p - j|)  for bin j in [0,128):
two adjacent nonzeros (1-frac, frac), and out-of-range corners drop out
automatically (matching grid_sample's zeros padding).

So for a batch of K points, the (y,x)-plane contribution at a fixed z-plane is
a dense matmul:   plane[y,x] += sum_k  wz_k * hat(y_k - y) * hat(x_k - x)
                             = (Wz.hatY)^T @ hatX

Sharding: the host (inside kernel(), as the sharding step) bins points by
z0 = floor(z_pix) into 129 groups (-1..127) and assigns core c the z-planes
[16c, 16c+15].  Core c processes groups 16c-1 .. 16c+15: group g contributes
to plane g (weight 1-fz) and plane g+1 (weight fz).  Each group is padded to a
fixed size so the single SPMD program works for every core; padded records use
far-away coordinates so every hat weight is exactly 0.

On device, per tile of 128 points (points live in partitions; record gives
per-partition scalar columns -y, -x, 1-fz, fz):
  ACT:  absd_y = Abs(iota + (-y));  absd_x = Abs(iota + (-x))   (exact: |u| is
        piecewise linear, and huge pad values only need to saturate >= 1)
  DVE:  hm_y = min(absd_y-1, 0) = -hat_y ;  X = min(absd_x-1, 0) = -hat_x (bf16)
        A = hm_y*(1-fz),  B = hm_y*fz     (bf16; A*X = +hat_y*hat_x*(1-fz))
  PE :  pair[x, 0:128 | 128:256] += X^T @ [A | B]   (PSUM f32 accumulate)

Plane p is then  pairs[p][:, :128] + pairs[p-1][:, 128:];  Huber uses the
branch-free identity  huber(d) = m*(|d| - m/2),  m = min(|d|, 1).
Per-core output is a [128] vector of partial sums; host adds them up.
"""

import os
import sys
import numpy as np

sys.path.insert(0, "/opt/trn_rl_repo")

from contextlib import ExitStack

import concourse.bass as bass
import concourse.tile as tile
from concourse import bacc, mybir
from concourse import dve_ops as _dvo
from concourse.bass_utils import run_bass_kernel_spmd
from concourse.dve_table_gen import dve_ver_for
from concourse.dve_spec import C0, C1, One, Spec, Src0, lower as _dve_lower, maxx, relu
from concourse.dve_uop import DveOpSpec

F32 = mybir.dt.float32
BF16 = mybir.dt.bfloat16

D = H = W = 128
N_CORES = 8
NG = 17            # groups per core: z0 in [16c-1, 16c+15]
NT_MIN = 8         # tiles of 128 points per group; actual NT sized from data

NREC = 4
_PAD_REC = np.array([40960.0, 40960.0, 0.0, 0.0], dtype=np.float32)


def _hat_ref(in0, in1, s0, s1, imm2):
    ad = np.abs(in0.astype(np.float32) - s0)
    return (np.maximum(1.0 - ad, 0.0) * s1).astype(np.float32)


def _make_hat_op():
    """Register a custom DVE op: out = relu(1 - |Src0 - s0|) * s1.

    One instruction builds a scaled trilinear hat row from the iota tile
    (s0 = point coord, s1 = z-weight, both per-partition scalars)."""
    name = "HAT_SCALE_ANT"
    for op in _dvo.OPS:
        if op.name == name:
            return op
    d = Src0 - C0
    body = relu(One - maxx(d, -d)) * C1
    spec = Spec(body=body, reference=_hat_ref)
    _dvo._SUB_OPCODE_FOR_NAME[name] = max(_dvo._SUB_OPCODE_FOR_NAME.values()) + 1
    assert _dvo._SUB_OPCODE_FOR_NAME[name] < 0x20
    ver = dve_ver_for("TRN2")
    tmp = DveOpSpec(name=name, opcode=_dvo._SUB_OPCODE_FOR_NAME[name],
                    uops=_dve_lower(spec, ver=ver), rd1_en=False)
    op = _dvo.DveOp(name, spec, subdim=False, uops_sha={ver: tmp.sha(ver)})
    _dvo.OPS.append(op)
    _dvo.CUSTOM_DVE_SPECS[name] = spec
    return op


def _pix_groups(pts: np.ndarray):
    x = ((pts[:, 0] + 1.0) * np.float32(W) - 1.0) * np.float32(0.5)
    y = ((pts[:, 1] + 1.0) * np.float32(H) - 1.0) * np.float32(0.5)
    z = ((pts[:, 2] + 1.0) * np.float32(D) - 1.0) * np.float32(0.5)
    z0 = np.floor(z).astype(np.int32)
    keep = (z0 >= -1) & (z0 <= 127)
    return x[keep], y[keep], z[keep], z0[keep]


def _route_points(pts: np.ndarray, nt: int):
    """pts [N,3] float32 -> per-core [NG, 128, nt*NREC] float32 record arrays."""
    g_pad = nt * 128
    x, y, z, z0 = _pix_groups(pts)
    fz = z - z0.astype(np.float32)
    # record = [y, x, 1-fz, fz]; custom DVE op builds hat*scale per axis
    recs = np.stack([y, x, 1.0 - fz, fz], axis=1).astype(np.float32)

    order = np.argsort(z0, kind="stable")
    z0s = z0[order]
    recs_s = recs[order]
    counts = np.bincount(z0s + 1, minlength=129)
    if counts.max() > g_pad:
        raise RuntimeError(f"group overflow: {counts.max()} > {g_pad}")
    starts = np.concatenate([[0], np.cumsum(counts)])

    glob = np.empty((129, g_pad, NREC), dtype=np.float32)
    glob[:] = _PAD_REC
    for g in range(129):
        n = counts[g]
        if n:
            glob[g, :n] = recs_s[starts[g]:starts[g] + n]

    per_core = []
    for c in range(N_CORES):
        arr = glob[16 * c: 16 * c + NG]                       # [NG, g_pad, NREC]
        arr = arr.reshape(NG, nt, 128, NREC).transpose(0, 2, 1, 3)
        per_core.append(np.ascontiguousarray(arr.reshape(NG, 128, nt * NREC)))
    return per_core


def build_bass(ng, nt):
    nc = bacc.Bacc(
        "TRN2", target_bir_lowering=False, debug=False, num_devices=N_CORES)
    recs_p = nc.declare_dram_parameter("recs_pred", [ng, 128, nt * NREC], F32, isOutput=False)
    recs_g = nc.declare_dram_parameter("recs_gt", [ng, 128, nt * NREC], F32, isOutput=False)
    out_part = nc.declare_dram_parameter("partials", [128, 1], F32, isOutput=True)

    hat_op = _make_hat_op()
    iota_np = np.tile(np.arange(128, dtype=np.float32), (128, 1))
    iota_dram = nc.inline_tensor(iota_np.astype(np.float32), "iota_const")

    recs_in = {0: recs_p, 1: recs_g}

    with tile.TileContext(nc) as tc, ExitStack() as ctx:
        const_pool = ctx.enter_context(tc.tile_pool(name="const", bufs=1))
        rec_pool = ctx.enter_context(tc.tile_pool(name="recs", bufs=4))
        work_pool = ctx.enter_context(tc.tile_pool(name="work", bufs=4))
        ab_pool = ctx.enter_context(tc.tile_pool(name="ab", bufs=4))
        x_pool = ctx.enter_context(tc.tile_pool(name="xt", bufs=4))
        flush_pool = ctx.enter_context(tc.tile_pool(name="flush", bufs=2))
        acc_pool = ctx.enter_context(tc.tile_pool(name="acc", bufs=1))
        psum_pools = {
            0: ctx.enter_context(tc.tile_pool(name="psum_p", bufs=3, space="PSUM")),
            1: ctx.enter_context(tc.tile_pool(name="psum_g", bufs=3, space="PSUM")),
        }

        iota_sb = const_pool.tile([128, 128], F32)
        nc.sync.dma_start(iota_sb[:], iota_dram[:])

        acc = acc_pool.tile([128, 128], F32)
        nc.vector.memset(acc[:], 0.0)

        pairs = {0: {}, 1: {}}  # grid -> local group idx -> psum pair tile

        for gi in range(ng):
            for grid in (0, 1):
                rec = rec_pool.tile([128, nt * NREC], F32, tag="rec")
                nc.sync.dma_start(rec[:], recs_in[grid][gi])

                pair = psum_pools[grid].tile([128, 256], F32, tag="pair")
                pairs[grid][gi] = pair

                for t in range(nt):
                    y_col = rec[:, NREC * t + 0: NREC * t + 1]
                    x_col = rec[:, NREC * t + 1: NREC * t + 2]
                    fa_col = rec[:, NREC * t + 2: NREC * t + 3]
                    fb_col = rec[:, NREC * t + 3: NREC * t + 4]

                    ab = ab_pool.tile([128, 256], BF16, tag="ab")
                    nc.vector._custom_dve(
                        hat_op, out=ab[:, 0:128], in0=iota_sb[:], s0=y_col, s1=fa_col)
                    nc.vector._custom_dve(
                        hat_op, out=ab[:, 128:256], in0=iota_sb[:], s0=y_col, s1=fb_col)
                    xt = x_pool.tile([128, 128], BF16, tag="xt")
                    nc.vector._custom_dve(
                        hat_op, out=xt[:], in0=iota_sb[:], s0=x_col, s1=1.0)

                    nc.tensor.matmul(
                        pair[:], xt[:], ab[:],
                        start=(t == 0), stop=(t == nt - 1))

            # flush local plane gi (valid for gi >= 1)
            if gi >= 1:
                pP1, pP0 = pairs[0][gi], pairs[0][gi - 1]
                pG1, pG0 = pairs[1][gi], pairs[1][gi - 1]
                c1 = flush_pool.tile([128, 128], F32, tag="c1")
                nc.scalar.copy(c1[:], pP1[:, 0:128])
                t1 = flush_pool.tile([128, 128], F32, tag="t1")
                nc.vector.tensor_tensor(t1[:], c1[:], pP0[:, 128:256], mybir.AluOpType.add)
                c2 = flush_pool.tile([128, 128], F32, tag="c2")
                nc.scalar.copy(c2[:], pG1[:, 0:128])
                t2 = flush_pool.tile([128, 128], F32, tag="t2")
                nc.vector.tensor_tensor(t2[:], c2[:], pG0[:, 128:256], mybir.AluOpType.add)
                d = flush_pool.tile([128, 128], F32, tag="d")
                nc.vector.tensor_tensor(d[:], t1[:], t2[:], mybir.AluOpType.subtract)
                nd = flush_pool.tile([128, 128], F32, tag="nd")
                nc.vector.tensor_scalar(
                    nd[:], d[:], -1.0, None, mybir.AluOpType.mult)
                a = flush_pool.tile([128, 128], F32, tag="a")
                nc.vector.tensor_tensor(a[:], d[:], nd[:], mybir.AluOpType.max)
                m = flush_pool.tile([128, 128], F32, tag="m")
                nc.vector.tensor_scalar(
                    m[:], a[:], 1.0, None, mybir.AluOpType.min)
                mh = flush_pool.tile([128, 128], F32, tag="mh")
                nc.vector.tensor_scalar(
                    mh[:], m[:], 0.5, None, mybir.AluOpType.mult)
                s = flush_pool.tile([128, 128], F32, tag="s")
                nc.vector.tensor_tensor(s[:], a[:], mh[:], mybir.AluOpType.subtract)
                h = flush_pool.tile([128, 128], F32, tag="h")
                nc.vector.tensor_tensor(h[:], m[:], s[:], mybir.AluOpType.mult)
                nc.vector.tensor_tensor(acc[:], acc[:], h[:], mybir.AluOpType.add)

        red = acc_pool.tile([128, 1], F32)
        nc.vector.tensor_reduce(red[:], acc[:], mybir.AxisListType.X, mybir.AluOpType.add)
        nc.sync.dma_start(out_part[:], red[:])

    nc.compile()
    return nc


_NC_CACHE = {}


def kernel(registration_pred, registration_gt, coords):
    coords = np.asarray(coords, dtype=np.float32)
    registration_pred = np.asarray(registration_pred, dtype=np.float32)
    registration_gt = np.asarray(registration_gt, dtype=np.float32)

    pred_pts = (coords + registration_pred).reshape(-1, 3).astype(np.float32)
    gt_pts = (coords + registration_gt).reshape(-1, 3).astype(np.float32)

    nt = NT_MIN
    for pts in (pred_pts, gt_pts):
        z0 = _pix_groups(pts)[3]
        nt = max(nt, int(-(-np.bincount(z0 + 1, minlength=129).max() // 128)))

    pred_cores = _route_points(pred_pts, nt)
    gt_cores = _route_points(gt_pts, nt)

    key = (NG, nt)
    if key not in _NC_CACHE:
        _NC_CACHE[key] = build_bass(NG, nt)
    nc = _NC_CACHE[key]

    in_maps = [
        {"recs_pred": pred_cores[c], "recs_gt": gt_cores[c]}
        for c in range(N_CORES)
    ]
    res = run_bass_kernel_spmd(nc, in_maps, list(range(N_CORES)))
    total = np.float64(0.0)
    for c in range(N_CORES):
        total += np.asarray(res.results[c]["partials"], dtype=np.float64).sum()
    return np.float32(total)


if __name__ == "__main__":
    rng = np.random.default_rng(0)
    n = 5000
    coords = rng.uniform(-0.95, 0.95, (1, n, 3)).astype(np.float32)
    rp = (0.02 * rng.standard_normal((1, n, 3))).astype(np.float32)
    rg = (0.02 * rng.standard_normal((1, n, 3))).astype(np.float32)
    print(kernel(rp, rg, coords))


# revision 20
# speedup vs baseline: 1.0822x; 1.0822x over previous
"""Trilinear scatter-add (splat) + Huber loss kernel for Trainium2, 8 NeuronCores.

Strategy
--------
reference computes:  huber_sum(splat(coords+pred) - splat(coords+gt)) over a
128^3 grid with trilinear weights and vals=1.

Key identity: the trilinear corner weights of a point with pixel coordinate p
along one axis are exactly  hat_j(p) = relu(1 - |p - j|)  for bin j in [0,128):
two adjacent nonzeros (1-frac, frac), and out-of-range corners drop out
automatically (matching grid_sample's zeros padding).

So for a batch of K points, the (y,x)-plane contribution at a fixed z-plane is
a dense matmul:   plane[y,x] += sum_k  wz_k * hat(y_k - y) * hat(x_k - x)
                             = (Wz.hatY)^T @ hatX

Sharding: the host (inside kernel(), as the sharding step) bins points by
z0 = floor(z_pix) into 129 groups (-1..127) and assigns core c the z-planes
[16c, 16c+15].  Core c processes groups 16c-1 .. 16c+15: group g contributes
to plane g (weight 1-fz) and plane g+1 (weight fz).  Each group is padded to a
fixed size so the single SPMD program works for every core; padded records use
far-away coordinates so every hat weight is exactly 0.

On device, per tile of 128 points (points live in partitions; record gives
per-partition scalar columns y, x, 1-fz, fz).  A runtime-registered custom
DVE op (HAT_SCALE_ANT: out = relu(1 - |Src0 - s0|) * s1, 6 ALU stages) builds
each scaled hat row in ONE instruction from the shared iota tile:
  DVE:  A = hat(iota - y)*(1-fz),  B = hat(iota - y)*fz,  X = hat(iota - x)
        (bf16 outs; 3 instructions per tile, ACT engine left entirely free)
  PE :  pair[x, 0:128 | 128:256] += X^T @ [A | B]   (PSUM f32 accumulate)

Plane p is then  pairs[p][:, :128] + pairs[p-1][:, 128:];  Huber uses the
branch-free identity  huber(d) = m*(|d| - m/2),  m = min(|d|, 1).
Per-core output is a [128] vector of partial sums; host adds them up.
"""

import os
import sys
import numpy as np

sys.path.insert(0, "/opt/trn_rl_repo")

from contextlib import ExitStack

import concourse.bass as bass
import concourse.tile as tile
from concourse import bacc, mybir
from concourse import dve_ops as _dvo
from concourse.bass_utils import run_bass_kernel_spmd
from concourse.dve_table_gen import dve_ver_for
from concourse.dve_spec import C0, C1, One, Spec, Src0, lower as _dve_lower, maxx, relu
from concourse.dve_uop import DveOpSpec

F32 = mybir.dt.float32
BF16 = mybir.dt.bfloat16

D = H = W = 128
N_CORES = 8
NG = 17            # groups per core: z0 in [16c-1, 16c+15]
NT_MIN = 8         # tiles of 128 points per group; actual NT sized from data

NREC = 4
_PAD_REC = np.array([40960.0, 40960.0, 0.0, 0.0], dtype=np.float32)


def _hat_ref(in0, in1, s0, s1, imm2):
    ad = np.abs(in0.astype(np.float32) - s0)
    return (np.maximum(1.0 - ad, 0.0) * s1).astype(np.float32)


def _make_hat_op():
    """Register a custom DVE op: out = relu(1 - |Src0 - s0|) * s1.

    One instruction builds a scaled trilinear hat row from the iota tile
    (s0 = point coord, s1 = z-weight, both per-partition scalars)."""
    name = "HAT_SCALE_ANT"
    for op in _dvo.OPS:
        if op.name == name:
            return op
    d = Src0 - C0
    body = relu(One - maxx(d, -d)) * C1
    spec = Spec(body=body, reference=_hat_ref)
    _dvo._SUB_OPCODE_FOR_NAME[name] = max(_dvo._SUB_OPCODE_FOR_NAME.values()) + 1
    assert _dvo._SUB_OPCODE_FOR_NAME[name] < 0x20
    ver = dve_ver_for("TRN2")
    tmp = DveOpSpec(name=name, opcode=_dvo._SUB_OPCODE_FOR_NAME[name],
                    uops=_dve_lower(spec, ver=ver), rd1_en=False)
    op = _dvo.DveOp(name, spec, subdim=False, uops_sha={ver: tmp.sha(ver)})
    _dvo.OPS.append(op)
    _dvo.CUSTOM_DVE_SPECS[name] = spec
    return op


def _pix_groups(pts: np.ndarray):
    x = ((pts[:, 0] + 1.0) * np.float32(W) - 1.0) * np.float32(0.5)
    y = ((pts[:, 1] + 1.0) * np.float32(H) - 1.0) * np.float32(0.5)
    z = ((pts[:, 2] + 1.0) * np.float32(D) - 1.0) * np.float32(0.5)
    z0 = np.floor(z).astype(np.int32)
    keep = (z0 >= -1) & (z0 <= 127)
    return x[keep], y[keep], z[keep], z0[keep]


def _subgroup_recs(pts: np.ndarray):
    """-> (recs [M,4], key [M]) with key = (z0+1)*2 + yhalf in [0, 258).
    Records store y LOCAL to the 64-bin half; y0==63 straddlers are
    duplicated into the upper half (their hat picks up only bin 64 there)."""
    x, y, z, z0 = _pix_groups(pts)
    fz = z - z0.astype(np.float32)
    y0 = np.floor(y).astype(np.int32)
    yh = (y0 >= 64).astype(np.int32)
    yl = y - 64.0 * yh.astype(np.float32)
    recs = np.stack([yl, x, 1.0 - fz, fz], axis=1).astype(np.float32)
    key = (z0 + 1) * 2 + yh
    dup = y0 == 63
    recs_d = np.stack([y[dup] - 64.0, x[dup], 1.0 - fz[dup], fz[dup]],
                      axis=1).astype(np.float32)
    key_d = (z0[dup] + 1) * 2 + 1
    return (np.concatenate([recs, recs_d]),
            np.concatenate([key, key_d]).astype(np.int64))


def _route_points(pts: np.ndarray, nt: int):
    """pts [N,3] float32 -> per-core [2*NG, 128, nt*NREC] record arrays."""
    g_pad = nt * 128
    recs, key = _subgroup_recs(pts)

    order = np.argsort(key, kind="stable")
    keys = key[order]
    recs_s = recs[order]
    counts = np.bincount(keys, minlength=258)
    if counts.max() > g_pad:
        raise RuntimeError(f"group overflow: {counts.max()} > {g_pad}")
    starts = np.concatenate([[0], np.cumsum(counts)])

    glob = np.empty((258, g_pad, NREC), dtype=np.float32)
    glob[:] = _PAD_REC
    for g in range(258):
        n = counts[g]
        if n:
            glob[g, :n] = recs_s[starts[g]:starts[g] + n]

    ns = 2 * NG
    per_core = []
    for c in range(N_CORES):
        arr = glob[32 * c: 32 * c + ns]                       # [2NG, g_pad, NREC]
        arr = arr.reshape(ns, nt, 128, NREC).transpose(0, 2, 1, 3)
        per_core.append(np.ascontiguousarray(arr.reshape(ns, 128, nt * NREC)))
    return per_core


def build_bass(ng, nt):
    nc = bacc.Bacc(
        "TRN2", target_bir_lowering=False, debug=False, num_devices=N_CORES)
    recs_p = nc.declare_dram_parameter("recs_pred", [ng, 128, nt * NREC], F32, isOutput=False)
    recs_g = nc.declare_dram_parameter("recs_gt", [ng, 128, nt * NREC], F32, isOutput=False)
    out_part = nc.declare_dram_parameter("partials", [128, 1], F32, isOutput=True)

    hat_op = _make_hat_op()
    iota_np = np.tile(np.arange(128, dtype=np.float32), (128, 1))
    iota_dram = nc.inline_tensor(iota_np.astype(np.float32), "iota_const")

    recs_in = {0: recs_p, 1: recs_g}

    with tile.TileContext(nc) as tc, ExitStack() as ctx:
        const_pool = ctx.enter_context(tc.tile_pool(name="const", bufs=1))
        rec_pool = ctx.enter_context(tc.tile_pool(name="recs", bufs=4))
        work_pool = ctx.enter_context(tc.tile_pool(name="work", bufs=4))
        ab_pool = ctx.enter_context(tc.tile_pool(name="ab", bufs=4))
        x_pool = ctx.enter_context(tc.tile_pool(name="xt", bufs=4))
        flush_pool = ctx.enter_context(tc.tile_pool(name="flush", bufs=2))
        acc_pool = ctx.enter_context(tc.tile_pool(name="acc", bufs=1))
        psum_pools = {
            0: ctx.enter_context(tc.tile_pool(name="psum_p", bufs=4, space="PSUM")),
            1: ctx.enter_context(tc.tile_pool(name="psum_g", bufs=4, space="PSUM")),
        }

        iota_sb = const_pool.tile([128, 128], F32)
        nc.sync.dma_start(iota_sb[:], iota_dram[:])

        acc = acc_pool.tile([128, 128], F32)
        nc.vector.memset(acc[:], 0.0)

        pairs = {0: {}, 1: {}}  # grid -> local group idx -> psum pair tile

        for s in range(ng):            # slab s = 2*gi + yhalf
            for grid in (0, 1):
                rec = rec_pool.tile([128, nt * NREC], F32, tag="rec")
                nc.sync.dma_start(rec[:], recs_in[grid][s])

                pair = psum_pools[grid].tile([128, 128], F32, tag="pair")
                pairs[grid][s] = pair

                for t in range(nt):
                    y_col = rec[:, NREC * t + 0: NREC * t + 1]
                    x_col = rec[:, NREC * t + 1: NREC * t + 2]
                    fa_col = rec[:, NREC * t + 2: NREC * t + 3]
                    fb_col = rec[:, NREC * t + 3: NREC * t + 4]

                    ab = ab_pool.tile([128, 128], BF16, tag="ab")
                    nc.vector._custom_dve(
                        hat_op, out=ab[:, 0:64], in0=iota_sb[:, 0:64],
                        s0=y_col, s1=fa_col)
                    nc.vector._custom_dve(
                        hat_op, out=ab[:, 64:128], in0=iota_sb[:, 0:64],
                        s0=y_col, s1=fb_col)
                    xt = x_pool.tile([128, 128], BF16, tag="xt")
                    nc.vector._custom_dve(
                        hat_op, out=xt[:], in0=iota_sb[:], s0=x_col, s1=1.0)

                    nc.tensor.matmul(
                        pair[:], xt[:], ab[:],
                        start=(t == 0), stop=(t == nt - 1))

            # flush local plane lp once slabs 2lp, 2lp+1 (A) and 2lp-2, 2lp-1
            # (B) are complete, i.e. at the end of odd slab s = 2lp+1, lp >= 1
            if s % 2 == 1 and s >= 3:
                lp = (s - 1) // 2
                d = flush_pool.tile([128, 128], F32, tag="d")
                for yh in (0, 1):
                    sA = 2 * lp + yh
                    sB = 2 * (lp - 1) + yh
                    cP = flush_pool.tile([128, 64], F32, tag="cP")
                    nc.scalar.copy(cP[:], pairs[0][sA][:, 0:64])
                    tP = flush_pool.tile([128, 64], F32, tag="tP")
                    nc.vector.tensor_tensor(
                        tP[:], cP[:], pairs[0][sB][:, 64:128], mybir.AluOpType.add)
                    cG = flush_pool.tile([128, 64], F32, tag="cG")
                    nc.scalar.copy(cG[:], pairs[1][sA][:, 0:64])
                    tG = flush_pool.tile([128, 64], F32, tag="tG")
                    nc.vector.tensor_tensor(
                        tG[:], cG[:], pairs[1][sB][:, 64:128], mybir.AluOpType.add)
                    nc.vector.tensor_tensor(
                        d[:, 64 * yh: 64 * yh + 64], tP[:], tG[:],
                        mybir.AluOpType.subtract)
                nd = flush_pool.tile([128, 128], F32, tag="nd")
                nc.vector.tensor_scalar(
                    nd[:], d[:], -1.0, None, mybir.AluOpType.mult)
                a = flush_pool.tile([128, 128], F32, tag="a")
                nc.vector.tensor_tensor(a[:], d[:], nd[:], mybir.AluOpType.max)
                m = flush_pool.tile([128, 128], F32, tag="m")
                nc.vector.tensor_scalar(
                    m[:], a[:], 1.0, None, mybir.AluOpType.min)
                mh = flush_pool.tile([128, 128], F32, tag="mh")
                nc.vector.tensor_scalar(
                    mh[:], m[:], 0.5, None, mybir.AluOpType.mult)
                sdiff = flush_pool.tile([128, 128], F32, tag="sdiff")
                nc.vector.tensor_tensor(sdiff[:], a[:], mh[:], mybir.AluOpType.subtract)
                h = flush_pool.tile([128, 128], F32, tag="h")
                nc.vector.tensor_tensor(h[:], m[:], sdiff[:], mybir.AluOpType.mult)
                nc.vector.tensor_tensor(acc[:], acc[:], h[:], mybir.AluOpType.add)

        red = acc_pool.tile([128, 1], F32)
        nc.vector.tensor_reduce(red[:], acc[:], mybir.AxisListType.X, mybir.AluOpType.add)
        nc.sync.dma_start(out_part[:], red[:])

    nc.compile()
    return nc


_NC_CACHE = {}


def kernel(registration_pred, registration_gt, coords):
    coords = np.asarray(coords, dtype=np.float32)
    registration_pred = np.asarray(registration_pred, dtype=np.float32)
    registration_gt = np.asarray(registration_gt, dtype=np.float32)

    pred_pts = (coords + registration_pred).reshape(-1, 3).astype(np.float32)
    gt_pts = (coords + registration_gt).reshape(-1, 3).astype(np.float32)

    nt = NT_MIN
    for pts in (pred_pts, gt_pts):
        key = _subgroup_recs(pts)[1]
        nt = max(nt, int(-(-np.bincount(key, minlength=258).max() // 128)))

    pred_cores = _route_points(pred_pts, nt)
    gt_cores = _route_points(gt_pts, nt)

    key = (2 * NG, nt)
    if key not in _NC_CACHE:
        _NC_CACHE[key] = build_bass(2 * NG, nt)
    nc = _NC_CACHE[key]

    in_maps = [
        {"recs_pred": pred_cores[c], "recs_gt": gt_cores[c]}
        for c in range(N_CORES)
    ]
    res = run_bass_kernel_spmd(nc, in_maps, list(range(N_CORES)))
    total = np.float64(0.0)
    for c in range(N_CORES):
        total += np.asarray(res.results[c]["partials"], dtype=np.float64).sum()
    return np.float32(total)


if __name__ == "__main__":
    rng = np.random.default_rng(0)
    n = 5000
    coords = rng.uniform(-0.95, 0.95, (1, n, 3)).astype(np.float32)
    rp = (0.02 * rng.standard_normal((1, n, 3))).astype(np.float32)
    rg = (0.02 * rng.standard_normal((1, n, 3))).astype(np.float32)
    print(kernel(rp, rg, coords))


# revision 24
# speedup vs baseline: 1.2388x; 1.1448x over previous
"""Trilinear scatter-add (splat) + Huber loss kernel for Trainium2, 8 NeuronCores.

Strategy
--------
reference computes:  huber_sum(splat(coords+pred) - splat(coords+gt)) over a
128^3 grid with trilinear weights and vals=1.

Key identity: the trilinear corner weights of a point with pixel coordinate p
along one axis are exactly  hat_j(p) = relu(1 - |p - j|)  for bin j in [0,128):
two adjacent nonzeros (1-frac, frac), and out-of-range corners drop out
automatically (matching grid_sample's zeros padding).

So for a batch of K points, the (y,x)-plane contribution at a fixed z-plane is
a dense matmul:   plane[y,x] += sum_k  wz_k * hat(y_k - y) * hat(x_k - x)
                             = (Wz.hatY)^T @ hatX

Sharding: the host (inside kernel(), as the sharding step) bins points by
(z0, y-half) into 258 subgroups and assigns core c the z-planes [16c, 16c+15].
Core c processes slabs s = 2*gi + yh for local groups gi = 0..16 (global z0 =
16c-1+gi): slab contributes 64 y-bins of plane z0 (weight 1-fz) and of plane
z0+1 (weight fz).  y0==63 straddlers are duplicated into the upper half.
Subgroups are padded to a fixed size so one SPMD program fits all cores;
padded records use far-away coordinates so every hat weight is exactly 0.

On device, per tile of 128 points (points live in partitions; record gives
per-partition scalar columns y, x, 1-fz, fz).  A runtime-registered custom
DVE op (HAT_SCALE_ANT: out = relu(1 - |Src0 - s0|) * s1, 6 ALU stages) builds
each scaled hat row in ONE instruction from the shared iota tile:
  DVE:  A = hat(iota - y)*(1-fz),  B = hat(iota - y)*fz,  X = hat(iota - x)
        (bf16 outs; 3 instructions per tile, ACT engine left entirely free)
  PE :  pair[x, (A 0:64 | B 64:128)] += X^T @ [A | B]  (PSUM f32 accumulate;
        A/B are the 64-wide y-half hats scaled by the two z-weights)

Plane p, y-half yh is  pairs[2p+yh][:, :64] + pairs[2(p-1)+yh][:, 64:];
Huber uses the
branch-free identity  huber(d) = m*(|d| - m/2),  m = min(|d|, 1).
Per-core output is a [128] vector of partial sums; host adds them up.
"""

import os
import sys
import numpy as np

sys.path.insert(0, "/opt/trn_rl_repo")

from contextlib import ExitStack

import concourse.bass as bass
import concourse.tile as tile
from concourse import bacc, mybir
from concourse import dve_ops as _dvo
from concourse.bass_utils import run_bass_kernel_spmd
from concourse.dve_table_gen import dve_ver_for
from concourse.dve_spec import (
    C0, C1, C2, AluOp as _DveAluOp, Bin as _DveBin, Idx, One, Spec, Src0,
    lower as _dve_lower, maxx, relu, select as _dve_select)
from concourse.dve_uop import DveOpSpec

F32 = mybir.dt.float32
BF16 = mybir.dt.bfloat16

D = H = W = 128
N_CORES = 8
NG = 17            # groups per core: z0 in [16c-1, 16c+15]
NT_MIN = 8         # tiles of 128 points per group; actual NT sized from data

NREC = 4
_PAD_REC = np.array([40960.0, 40960.0, 0.0, 0.0], dtype=np.float32)


def _hat_ref(in0, in1, s0, s1, imm2):
    ad = np.abs(in0.astype(np.float32) - s0)
    return (np.maximum(1.0 - ad, 0.0) * s1).astype(np.float32)


def _hat_pair_ref(in0, in1, s0, s1, imm2):
    flat = in0.astype(np.float32).reshape(in0.shape[0], -1)
    hat = np.maximum(1.0 - np.abs(flat - s0), 0.0)
    idx = np.arange(flat.shape[1], dtype=np.float32)[None, :]
    sc = np.where(idx < imm2, s1, np.float32(1.0))
    return (hat * sc).astype(np.float32).reshape(in0.shape)


def _register_dve_op(name, spec):
    for op in _dvo.OPS:
        if op.name == name:
            return op
    _dvo._SUB_OPCODE_FOR_NAME[name] = max(_dvo._SUB_OPCODE_FOR_NAME.values()) + 1
    assert _dvo._SUB_OPCODE_FOR_NAME[name] < 0x20
    ver = dve_ver_for("TRN2")
    tmp = DveOpSpec(name=name, opcode=_dvo._SUB_OPCODE_FOR_NAME[name],
                    uops=_dve_lower(spec, ver=ver), rd1_en=False)
    op = _dvo.DveOp(name, spec, subdim=False, uops_sha={ver: tmp.sha(ver)})
    _dvo.OPS.append(op)
    _dvo.CUSTOM_DVE_SPECS[name] = spec
    return op


def _make_hat_op():
    """out = relu(1 - |Src0 - s0|) * s1: one scaled hat row per instruction."""
    ad = _DveBin(_DveAluOp.ABSOLUTE_DIFF, Src0, C0)
    return _register_dve_op(
        "HAT_SCALE_ANT",
        Spec(body=relu(One - ad) * C1, reference=_hat_ref))


def _make_hat_pair_op():
    """out[j] = relu(1 - |Src0 - s0|) * (j < imm2 ? s1 : 1).

    One instruction against a twice-repeated iota tile yields
    [hat*(1-fz) | hat]; the unscaled half is the A+B total, so the
    B (plane z0+1) contribution is recovered at flush as total - A.
    (select(.., C1, One-C1) would need a 9th ALU stage; One is a leaf.)"""
    ad = _DveBin(_DveAluOp.ABSOLUTE_DIFF, Src0, C0)
    hat = relu(One - ad)
    sc = _dve_select(_DveBin(_DveAluOp.IS_LT, Idx, C2), C1, One)
    return _register_dve_op(
        "HAT_PAIR_ANT", Spec(body=hat * sc, reference=_hat_pair_ref))


def _pix_groups(pts: np.ndarray):
    x = ((pts[:, 0] + 1.0) * np.float32(W) - 1.0) * np.float32(0.5)
    y = ((pts[:, 1] + 1.0) * np.float32(H) - 1.0) * np.float32(0.5)
    z = ((pts[:, 2] + 1.0) * np.float32(D) - 1.0) * np.float32(0.5)
    z0 = np.floor(z).astype(np.int32)
    keep = (z0 >= -1) & (z0 <= 127)
    return x[keep], y[keep], z[keep], z0[keep]


def _subgroup_recs(pts: np.ndarray):
    """-> (recs [M,4], key [M]) with key = (z0+1)*2 + yhalf in [0, 258).
    Records store y LOCAL to the 64-bin half; y0==63 straddlers are
    duplicated into the upper half (their hat picks up only bin 64 there)."""
    x, y, z, z0 = _pix_groups(pts)
    fz = z - z0.astype(np.float32)
    y0 = np.floor(y).astype(np.int32)
    yh = (y0 >= 64).astype(np.int32)
    yl = y - 64.0 * yh.astype(np.float32)
    recs = np.stack([yl, x, 1.0 - fz, fz], axis=1).astype(np.float32)
    key = (z0 + 1) * 2 + yh
    dup = y0 == 63
    recs_d = np.stack([y[dup] - 64.0, x[dup], 1.0 - fz[dup], fz[dup]],
                      axis=1).astype(np.float32)
    key_d = (z0[dup] + 1) * 2 + 1
    return (np.concatenate([recs, recs_d]),
            np.concatenate([key, key_d]).astype(np.int64))


def _route_points(pts: np.ndarray, nt: int):
    """pts [N,3] float32 -> per-core [2*NG, 128, nt*NREC] record arrays."""
    g_pad = nt * 128
    recs, key = _subgroup_recs(pts)

    order = np.argsort(key, kind="stable")
    keys = key[order]
    recs_s = recs[order]
    counts = np.bincount(keys, minlength=258)
    if counts.max() > g_pad:
        raise RuntimeError(f"group overflow: {counts.max()} > {g_pad}")
    starts = np.concatenate([[0], np.cumsum(counts)])

    glob = np.empty((258, g_pad, NREC), dtype=np.float32)
    glob[:] = _PAD_REC
    for g in range(258):
        n = counts[g]
        if n:
            glob[g, :n] = recs_s[starts[g]:starts[g] + n]

    ns = 2 * NG
    per_core = []
    for c in range(N_CORES):
        arr = glob[32 * c: 32 * c + ns]                       # [2NG, g_pad, NREC]
        arr = arr.reshape(ns, nt, 128, NREC).transpose(0, 2, 1, 3)
        per_core.append(np.ascontiguousarray(arr.reshape(ns, 128, nt * NREC)))
    return per_core


def build_bass(ng, nt):
    nc = bacc.Bacc(
        "TRN2", target_bir_lowering=False, debug=False, num_devices=N_CORES)
    recs_p = nc.declare_dram_parameter("recs_pred", [ng, 128, nt * NREC], F32, isOutput=False)
    recs_g = nc.declare_dram_parameter("recs_gt", [ng, 128, nt * NREC], F32, isOutput=False)
    out_part = nc.declare_dram_parameter("partials", [128, 1], F32, isOutput=True)

    hat_op = _make_hat_op()
    hat_pair_op = _make_hat_pair_op()
    iota_np = np.tile(np.arange(128, dtype=np.float32), (128, 1))
    iota_dram = nc.inline_tensor(iota_np.astype(np.float32), "iota_const")
    iota2_np = np.tile(np.concatenate([np.arange(64, dtype=np.float32)] * 2),
                       (128, 1))
    iota2_dram = nc.inline_tensor(iota2_np.astype(np.float32), "iota2_const")

    recs_in = {0: recs_p, 1: recs_g}

    with tile.TileContext(nc) as tc, ExitStack() as ctx:
        const_pool = ctx.enter_context(tc.tile_pool(name="const", bufs=1))
        rec_pool = ctx.enter_context(tc.tile_pool(name="recs", bufs=4))
        work_pool = ctx.enter_context(tc.tile_pool(name="work", bufs=4))
        ab_pool = ctx.enter_context(tc.tile_pool(name="ab", bufs=4))
        x_pool = ctx.enter_context(tc.tile_pool(name="xt", bufs=4))
        flush_pool = ctx.enter_context(tc.tile_pool(name="flush", bufs=2))
        acc_pool = ctx.enter_context(tc.tile_pool(name="acc", bufs=1))
        psum_pools = {
            0: ctx.enter_context(tc.tile_pool(name="psum_p", bufs=4, space="PSUM")),
            1: ctx.enter_context(tc.tile_pool(name="psum_g", bufs=4, space="PSUM")),
        }

        iota_sb = const_pool.tile([128, 128], F32)
        nc.sync.dma_start(iota_sb[:], iota_dram[:])
        iota2_sb = const_pool.tile([128, 128], F32, tag="iota2")
        nc.sync.dma_start(iota2_sb[:], iota2_dram[:])

        acc = acc_pool.tile([128, 128], F32)
        nc.vector.memset(acc[:], 0.0)

        pairs = {0: {}, 1: {}}  # grid -> local group idx -> psum pair tile

        for s in range(ng):            # slab s = 2*gi + yhalf
            for grid in (0, 1):
                rec = rec_pool.tile([128, nt * NREC], F32, tag="rec")
                nc.sync.dma_start(rec[:], recs_in[grid][s])

                pair = psum_pools[grid].tile([128, 128], F32, tag="pair")
                pairs[grid][s] = pair

                for t in range(nt):
                    y_col = rec[:, NREC * t + 0: NREC * t + 1]
                    x_col = rec[:, NREC * t + 1: NREC * t + 2]
                    fa_col = rec[:, NREC * t + 2: NREC * t + 3]
                    fb_col = rec[:, NREC * t + 3: NREC * t + 4]

                    ab = ab_pool.tile([128, 128], BF16, tag="ab")
                    nc.vector._custom_dve(
                        hat_pair_op, out=ab[:], in0=iota2_sb[:],
                        s0=y_col, s1=fa_col, imm2=64.0)
                    xt = x_pool.tile([128, 128], BF16, tag="xt")
                    nc.vector._custom_dve(
                        hat_op, out=xt[:], in0=iota_sb[:], s0=x_col, s1=1.0)

                    nc.tensor.matmul(
                        pair[:], xt[:], ab[:],
                        start=(t == 0), stop=(t == nt - 1))

            # flush local plane lp once slabs 2lp, 2lp+1 (A) and 2lp-2, 2lp-1
            # (B) are complete, i.e. at the end of odd slab s = 2lp+1, lp >= 1
            if s % 2 == 1 and s >= 3:
                lp = (s - 1) // 2
                d = flush_pool.tile([128, 128], F32, tag="d")
                for yh in (0, 1):
                    sA = 2 * lp + yh
                    sB = 2 * (lp - 1) + yh
                    # plane half = A(sA) + (total(sB) - A(sB))
                    cP = flush_pool.tile([128, 64], F32, tag="cP")
                    nc.scalar.copy(cP[:], pairs[0][sA][:, 0:64])
                    tP = flush_pool.tile([128, 64], F32, tag="tP")
                    nc.vector.tensor_tensor(
                        tP[:], cP[:], pairs[0][sB][:, 64:128], mybir.AluOpType.add)
                    tP2 = flush_pool.tile([128, 64], F32, tag="tP2")
                    nc.vector.tensor_tensor(
                        tP2[:], tP[:], pairs[0][sB][:, 0:64], mybir.AluOpType.subtract)
                    cG = flush_pool.tile([128, 64], F32, tag="cG")
                    nc.scalar.copy(cG[:], pairs[1][sA][:, 0:64])
                    tG = flush_pool.tile([128, 64], F32, tag="tG")
                    nc.vector.tensor_tensor(
                        tG[:], cG[:], pairs[1][sB][:, 64:128], mybir.AluOpType.add)
                    tG2 = flush_pool.tile([128, 64], F32, tag="tG2")
                    nc.vector.tensor_tensor(
                        tG2[:], tG[:], pairs[1][sB][:, 0:64], mybir.AluOpType.subtract)
                    nc.vector.tensor_tensor(
                        d[:, 64 * yh: 64 * yh + 64], tP2[:], tG2[:],
                        mybir.AluOpType.subtract)
                nd = flush_pool.tile([128, 128], F32, tag="nd")
                nc.vector.tensor_scalar(
                    nd[:], d[:], -1.0, None, mybir.AluOpType.mult)
                a = flush_pool.tile([128, 128], F32, tag="a")
                nc.vector.tensor_tensor(a[:], d[:], nd[:], mybir.AluOpType.max)
                m = flush_pool.tile([128, 128], F32, tag="m")
                nc.vector.tensor_scalar(
                    m[:], a[:], 1.0, None, mybir.AluOpType.min)
                mh = flush_pool.tile([128, 128], F32, tag="mh")
                nc.vector.tensor_scalar(
                    mh[:], m[:], 0.5, None, mybir.AluOpType.mult)
                sdiff = flush_pool.tile([128, 128], F32, tag="sdiff")
                nc.vector.tensor_tensor(sdiff[:], a[:], mh[:], mybir.AluOpType.subtract)
                h = flush_pool.tile([128, 128], F32, tag="h")
                nc.vector.tensor_tensor(h[:], m[:], sdiff[:], mybir.AluOpType.mult)
                nc.vector.tensor_tensor(acc[:], acc[:], h[:], mybir.AluOpType.add)

        red = acc_pool.tile([128, 1], F32)
        nc.vector.tensor_reduce(red[:], acc[:], mybir.AxisListType.X, mybir.AluOpType.add)
        nc.sync.dma_start(out_part[:], red[:])

    nc.compile()
    return nc


_NC_CACHE = {}


def kernel(registration_pred, registration_gt, coords):
    coords = np.asarray(coords, dtype=np.float32)
    registration_pred = np.asarray(registration_pred, dtype=np.float32)
    registration_gt = np.asarray(registration_gt, dtype=np.float32)

    pred_pts = (coords + registration_pred).reshape(-1, 3).astype(np.float32)
    gt_pts = (coords + registration_gt).reshape(-1, 3).astype(np.float32)

    nt = NT_MIN
    for pts in (pred_pts, gt_pts):
        key = _subgroup_recs(pts)[1]
        nt = max(nt, int(-(-np.bincount(key, minlength=258).max() // 128)))

    pred_cores = _route_points(pred_pts, nt)
    gt_cores = _route_points(gt_pts, nt)

    key = (2 * NG, nt)
    if key not in _NC_CACHE:
        _NC_CACHE[key] = build_bass(2 * NG, nt)
    nc = _NC_CACHE[key]

    in_maps = [
        {"recs_pred": pred_cores[c], "recs_gt": gt_cores[c]}
        for c in range(N_CORES)
    ]
    res = run_bass_kernel_spmd(nc, in_maps, list(range(N_CORES)))
    total = np.float64(0.0)
    for c in range(N_CORES):
        total += np.asarray(res.results[c]["partials"], dtype=np.float64).sum()
    return np.float32(total)


if __name__ == "__main__":
    rng = np.random.default_rng(0)
    n = 5000
    coords = rng.uniform(-0.95, 0.95, (1, n, 3)).astype(np.float32)
    rp = (0.02 * rng.standard_normal((1, n, 3))).astype(np.float32)
    rg = (0.02 * rng.standard_normal((1, n, 3))).astype(np.float32)
    print(kernel(rp, rg, coords))


# revision 26
# speedup vs baseline: 1.4301x; 1.1544x over previous
"""Trilinear scatter-add (splat) + Huber loss kernel for Trainium2, 8 NeuronCores.

Strategy
--------
reference computes:  huber_sum(splat(coords+pred) - splat(coords+gt)) over a
128^3 grid with trilinear weights and vals=1.

Key identity: the trilinear corner weights of a point with pixel coordinate p
along one axis are exactly  hat_j(p) = relu(1 - |p - j|)  for bin j in [0,128):
two adjacent nonzeros (1-frac, frac), and out-of-range corners drop out
automatically (matching grid_sample's zeros padding).

So for a batch of K points, the (y,x)-plane contribution at a fixed z-plane is
a dense matmul:   plane[y,x] += sum_k  wz_k * hat(y_k - y) * hat(x_k - x)
                             = (Wz.hatY)^T @ hatX

Sharding: the host (inside kernel(), as the sharding step) bins points by
(z0, y-half) into 258 subgroups and assigns core c the z-planes [16c, 16c+15].
Core c processes slabs s = 2*gi + yh for local groups gi = 0..16 (global z0 =
16c-1+gi): slab contributes 64 y-bins of plane z0 (weight 1-fz) and of plane
z0+1 (weight fz).  y0==63 straddlers are duplicated into the upper half.
Subgroups are padded to a fixed size so one SPMD program fits all cores;
padded records use far-away coordinates so every hat weight is exactly 0.

On device, per tile of 128 points (points live in partitions; record gives
per-partition scalar columns y, x, 1-fz, fz), TWO runtime-registered custom
DVE instructions build all weight rows (ABSOLUTE_DIFF fuses |Src0-s0|):
  HAT_PAIR_ANT:  ab[j] = relu(1-|iota2-y|) * (j<64 ? (1-fz) : 1)
                 -> [A | A+B total] for the two 64-wide y-half columns
  HAT_SCALE_ANT: xt = relu(1-|iota-x|)          (s1=1, 128-wide)
  PE :  pair[x, 0:128] += xt^T @ ab   (PSUM f32 accumulate)

Plane p, y-half yh = pairs[2p+yh][:, :64]            (A of group p)
                   + pairs[2(p-1)+yh][:, 64:]        (total of group p-1)
                   - pairs[2(p-1)+yh][:, :64]        (- A = B of group p-1);
Huber uses the branch-free identity  huber(d) = m*(|d| - m/2),  m = min(|d|, 1).
Per-core output is a [128] vector of partial sums; host adds them up.
"""

import os
import sys
import numpy as np

sys.path.insert(0, "/opt/trn_rl_repo")

from contextlib import ExitStack

import concourse.bass as bass
import concourse.tile as tile
from concourse import bacc, mybir
from concourse import dve_ops as _dvo
from concourse.bass_utils import run_bass_kernel_spmd
from concourse.dve_table_gen import dve_ver_for
from concourse.dve_spec import (
    C0, C1, C2, AluOp as _DveAluOp, Bin as _DveBin, Idx, One, Spec, Src0,
    lower as _dve_lower, maxx, relu, select as _dve_select)
from concourse.dve_uop import DveOpSpec

F32 = mybir.dt.float32
BF16 = mybir.dt.bfloat16

D = H = W = 128
N_CORES = 8
NG = 17            # groups per core: z0 in [16c-1, 16c+15]
NT_MIN = 8         # tiles of 128 points per group; actual NT sized from data

NREC = 4
_PAD_REC = np.array([40960.0, 40960.0, 0.0, -40960.0], dtype=np.float32)


def _hat_ref(in0, in1, s0, s1, imm2):
    ad = np.abs(in0.astype(np.float32) - s0)
    return (np.maximum(1.0 - ad, 0.0) * s1).astype(np.float32)


def _hat_pair_ref(in0, in1, s0, s1, imm2):
    flat = in0.astype(np.float32).reshape(in0.shape[0], -1)
    hat = np.maximum(1.0 - np.abs(flat - s0), 0.0)
    idx = np.arange(flat.shape[1], dtype=np.float32)[None, :]
    sc = np.where(idx < imm2, s1, np.float32(1.0))
    return (hat * sc).astype(np.float32).reshape(in0.shape)


def _register_dve_op(name, spec):
    for op in _dvo.OPS:
        if op.name == name:
            return op
    _dvo._SUB_OPCODE_FOR_NAME[name] = max(_dvo._SUB_OPCODE_FOR_NAME.values()) + 1
    assert _dvo._SUB_OPCODE_FOR_NAME[name] < 0x20
    ver = dve_ver_for("TRN2")
    tmp = DveOpSpec(name=name, opcode=_dvo._SUB_OPCODE_FOR_NAME[name],
                    uops=_dve_lower(spec, ver=ver), rd1_en=False)
    op = _dvo.DveOp(name, spec, subdim=False, uops_sha={ver: tmp.sha(ver)})
    _dvo.OPS.append(op)
    _dvo.CUSTOM_DVE_SPECS[name] = spec
    return op


def _make_hat_op():
    """out = relu(1 - |Src0 - s0|) * s1: one scaled hat row per instruction."""
    ad = _DveBin(_DveAluOp.ABSOLUTE_DIFF, Src0, C0)
    return _register_dve_op(
        "HAT_SCALE_ANT",
        Spec(body=relu(One - ad) * C1, reference=_hat_ref))


def _make_hat_pair_op():
    """out[j] = relu(1 - |Src0 - s0|) * (j < imm2 ? s1 : 1).

    One instruction against a twice-repeated iota tile yields
    [hat*(1-fz) | hat]; the unscaled half is the A+B total, so the
    B (plane z0+1) contribution is recovered at flush as total - A.
    (select(.., C1, One-C1) would need a 9th ALU stage; One is a leaf.)"""
    ad = _DveBin(_DveAluOp.ABSOLUTE_DIFF, Src0, C0)
    hat = relu(One - ad)
    sc = _dve_select(_DveBin(_DveAluOp.IS_LT, Idx, C2), C1, One)
    return _register_dve_op(
        "HAT_PAIR_ANT", Spec(body=hat * sc, reference=_hat_pair_ref))


def _pix_groups(pts: np.ndarray):
    x = ((pts[:, 0] + 1.0) * np.float32(W) - 1.0) * np.float32(0.5)
    y = ((pts[:, 1] + 1.0) * np.float32(H) - 1.0) * np.float32(0.5)
    z = ((pts[:, 2] + 1.0) * np.float32(D) - 1.0) * np.float32(0.5)
    z0 = np.floor(z).astype(np.int32)
    keep = (z0 >= -1) & (z0 <= 127)
    return x[keep], y[keep], z[keep], z0[keep]


def _subgroup_recs(pts: np.ndarray):
    """-> (recs [M,4], key [M]) with key = (z0+1)*2 + yhalf in [0, 258).
    Records store y LOCAL to the 64-bin half; y0==63 straddlers are
    duplicated into the upper half (their hat picks up only bin 64 there)."""
    x, y, z, z0 = _pix_groups(pts)
    fz = z - z0.astype(np.float32)
    y0 = np.floor(y).astype(np.int32)
    yh = (y0 >= 64).astype(np.int32)
    yl = y - 64.0 * yh.astype(np.float32)
    # [y_local, x, 1-fz, -x]; -x feeds the ACT Abs bias for the balanced xt
    recs = np.stack([yl, x, 1.0 - fz, -x], axis=1).astype(np.float32)
    key = (z0 + 1) * 2 + yh
    dup = y0 == 63
    recs_d = np.stack([y[dup] - 64.0, x[dup], 1.0 - fz[dup], -x[dup]],
                      axis=1).astype(np.float32)
    key_d = (z0[dup] + 1) * 2 + 1
    return (np.concatenate([recs, recs_d]),
            np.concatenate([key, key_d]).astype(np.int64))


def _route_points(pts: np.ndarray, nt: int):
    """pts [N,3] float32 -> per-core [2*NG, 128, nt*NREC] record arrays."""
    g_pad = nt * 128
    recs, key = _subgroup_recs(pts)

    order = np.argsort(key, kind="stable")
    keys = key[order]
    recs_s = recs[order]
    counts = np.bincount(keys, minlength=258)
    if counts.max() > g_pad:
        raise RuntimeError(f"group overflow: {counts.max()} > {g_pad}")
    starts = np.concatenate([[0], np.cumsum(counts)])

    glob = np.empty((258, g_pad, NREC), dtype=np.float32)
    glob[:] = _PAD_REC
    for g in range(258):
        n = counts[g]
        if n:
            glob[g, :n] = recs_s[starts[g]:starts[g] + n]

    ns = 2 * NG
    per_core = []
    for c in range(N_CORES):
        arr = glob[32 * c: 32 * c + ns]                       # [2NG, g_pad, NREC]
        arr = arr.reshape(ns, nt, 128, NREC).transpose(0, 2, 1, 3)
        per_core.append(np.ascontiguousarray(arr.reshape(ns, 128, nt * NREC)))
    return per_core


def build_bass(ng, nt):
    nc = bacc.Bacc(
        "TRN2", target_bir_lowering=False, debug=False, num_devices=N_CORES)
    recs_p = nc.declare_dram_parameter("recs_pred", [ng, 128, nt * NREC], F32, isOutput=False)
    recs_g = nc.declare_dram_parameter("recs_gt", [ng, 128, nt * NREC], F32, isOutput=False)
    out_part = nc.declare_dram_parameter("partials", [128, 1], F32, isOutput=True)

    hat_op = _make_hat_op()
    hat_pair_op = _make_hat_pair_op()
    iota_np = np.tile(np.arange(128, dtype=np.float32), (128, 1))
    iota_dram = nc.inline_tensor(iota_np.astype(np.float32), "iota_const")
    iota2_np = np.tile(np.concatenate([np.arange(64, dtype=np.float32)] * 2),
                       (128, 1))
    iota2_dram = nc.inline_tensor(iota2_np.astype(np.float32), "iota2_const")

    recs_in = {0: recs_p, 1: recs_g}

    with tile.TileContext(nc) as tc, ExitStack() as ctx:
        const_pool = ctx.enter_context(tc.tile_pool(name="const", bufs=1))
        rec_pool = ctx.enter_context(tc.tile_pool(name="recs", bufs=4))
        work_pool = ctx.enter_context(tc.tile_pool(name="work", bufs=4))
        ab_pool = ctx.enter_context(tc.tile_pool(name="ab", bufs=4))
        x_pool = ctx.enter_context(tc.tile_pool(name="xt", bufs=4))
        flush_pool = ctx.enter_context(tc.tile_pool(name="flush", bufs=2))
        acc_pool = ctx.enter_context(tc.tile_pool(name="acc", bufs=1))
        psum_pools = {
            0: ctx.enter_context(tc.tile_pool(name="psum_p", bufs=4, space="PSUM")),
            1: ctx.enter_context(tc.tile_pool(name="psum_g", bufs=4, space="PSUM")),
        }

        iota_sb = const_pool.tile([128, 128], F32)
        nc.sync.dma_start(iota_sb[:], iota_dram[:])
        iota2_sb = const_pool.tile([128, 128], F32, tag="iota2")
        nc.sync.dma_start(iota2_sb[:], iota2_dram[:])

        acc = acc_pool.tile([128, 128], F32)
        nc.vector.memset(acc[:], 0.0)

        pairs = {0: {}, 1: {}}  # grid -> local group idx -> psum pair tile

        for s in range(ng):            # slab s = 2*gi + yhalf
            for grid in (0, 1):
                rec = rec_pool.tile([128, nt * NREC], F32, tag="rec")
                nc.sync.dma_start(rec[:], recs_in[grid][s])

                pair = psum_pools[grid].tile([128, 128], F32, tag="pair")
                pairs[grid][s] = pair

                for t in range(nt):
                    y_col = rec[:, NREC * t + 0: NREC * t + 1]
                    x_col = rec[:, NREC * t + 1: NREC * t + 2]
                    fa_col = rec[:, NREC * t + 2: NREC * t + 3]
                    nx_col = rec[:, NREC * t + 3: NREC * t + 4]

                    ab = ab_pool.tile([128, 128], BF16, tag="ab")
                    nc.vector._custom_dve(
                        hat_pair_op, out=ab[:], in0=iota2_sb[:],
                        s0=y_col, s1=fa_col, imm2=64.0)
                    xt = x_pool.tile([128, 128], BF16, tag="xt")
                    if t % 7 < 4:
                        # ACT path: |iota - x| then relu(1 - absd); keeps the
                        # otherwise-idle ACT engine carrying ~4/7 of the xt work
                        absd = work_pool.tile([128, 128], F32, tag="absd")
                        nc.scalar.activation(
                            absd[:], iota_sb[:],
                            mybir.ActivationFunctionType.Abs, bias=nx_col)
                        nc.scalar.activation(
                            xt[:], absd[:],
                            mybir.ActivationFunctionType.Relu,
                            bias=1.0, scale=-1.0)
                    else:
                        nc.vector._custom_dve(
                            hat_op, out=xt[:], in0=iota_sb[:], s0=x_col, s1=1.0)

                    nc.tensor.matmul(
                        pair[:], xt[:], ab[:],
                        start=(t == 0), stop=(t == nt - 1))

            # flush local plane lp once slabs 2lp, 2lp+1 (A) and 2lp-2, 2lp-1
            # (B) are complete, i.e. at the end of odd slab s = 2lp+1, lp >= 1
            if s % 2 == 1 and s >= 3:
                lp = (s - 1) // 2
                d = flush_pool.tile([128, 128], F32, tag="d")
                for yh in (0, 1):
                    sA = 2 * lp + yh
                    sB = 2 * (lp - 1) + yh
                    # plane half = A(sA) + (total(sB) - A(sB))
                    cP = flush_pool.tile([128, 64], F32, tag="cP")
                    nc.scalar.copy(cP[:], pairs[0][sA][:, 0:64])
                    tP = flush_pool.tile([128, 64], F32, tag="tP")
                    nc.vector.tensor_tensor(
                        tP[:], cP[:], pairs[0][sB][:, 64:128], mybir.AluOpType.add)
                    tP2 = flush_pool.tile([128, 64], F32, tag="tP2")
                    nc.vector.tensor_tensor(
                        tP2[:], tP[:], pairs[0][sB][:, 0:64], mybir.AluOpType.subtract)
                    cG = flush_pool.tile([128, 64], F32, tag="cG")
                    nc.scalar.copy(cG[:], pairs[1][sA][:, 0:64])
                    tG = flush_pool.tile([128, 64], F32, tag="tG")
                    nc.vector.tensor_tensor(
                        tG[:], cG[:], pairs[1][sB][:, 64:128], mybir.AluOpType.add)
                    tG2 = flush_pool.tile([128, 64], F32, tag="tG2")
                    nc.vector.tensor_tensor(
                        tG2[:], tG[:], pairs[1][sB][:, 0:64], mybir.AluOpType.subtract)
                    nc.vector.tensor_tensor(
                        d[:, 64 * yh: 64 * yh + 64], tP2[:], tG2[:],
                        mybir.AluOpType.subtract)
                nd = flush_pool.tile([128, 128], F32, tag="nd")
                nc.vector.tensor_scalar(
                    nd[:], d[:], -1.0, None, mybir.AluOpType.mult)
                a = flush_pool.tile([128, 128], F32, tag="a")
                nc.vector.tensor_tensor(a[:], d[:], nd[:], mybir.AluOpType.max)
                m = flush_pool.tile([128, 128], F32, tag="m")
                nc.vector.tensor_scalar(
                    m[:], a[:], 1.0, None, mybir.AluOpType.min)
                mh = flush_pool.tile([128, 128], F32, tag="mh")
                nc.vector.tensor_scalar(
                    mh[:], m[:], 0.5, None, mybir.AluOpType.mult)
                sdiff = flush_pool.tile([128, 128], F32, tag="sdiff")
                nc.vector.tensor_tensor(sdiff[:], a[:], mh[:], mybir.AluOpType.subtract)
                h = flush_pool.tile([128, 128], F32, tag="h")
                nc.vector.tensor_tensor(h[:], m[:], sdiff[:], mybir.AluOpType.mult)
                nc.vector.tensor_tensor(acc[:], acc[:], h[:], mybir.AluOpType.add)

        red = acc_pool.tile([128, 1], F32)
        nc.vector.tensor_reduce(red[:], acc[:], mybir.AxisListType.X, mybir.AluOpType.add)
        nc.sync.dma_start(out_part[:], red[:])

    nc.compile()
    return nc


_NC_CACHE = {}


def kernel(registration_pred, registration_gt, coords):
    coords = np.asarray(coords, dtype=np.float32)
    registration_pred = np.asarray(registration_pred, dtype=np.float32)
    registration_gt = np.asarray(registration_gt, dtype=np.float32)

    pred_pts = (coords + registration_pred).reshape(-1, 3).astype(np.float32)
    gt_pts = (coords + registration_gt).reshape(-1, 3).astype(np.float32)

    nt = NT_MIN
    for pts in (pred_pts, gt_pts):
        key = _subgroup_recs(pts)[1]
        nt = max(nt, int(-(-np.bincount(key, minlength=258).max() // 128)))

    pred_cores = _route_points(pred_pts, nt)
    gt_cores = _route_points(gt_pts, nt)

    key = (2 * NG, nt)
    if key not in _NC_CACHE:
        _NC_CACHE[key] = build_bass(2 * NG, nt)
    nc = _NC_CACHE[key]

    in_maps = [
        {"recs_pred": pred_cores[c], "recs_gt": gt_cores[c]}
        for c in range(N_CORES)
    ]
    res = run_bass_kernel_spmd(nc, in_maps, list(range(N_CORES)))
    total = np.float64(0.0)
    for c in range(N_CORES):
        total += np.asarray(res.results[c]["partials"], dtype=np.float64).sum()
    return np.float32(total)


if __name__ == "__main__":
    rng = np.random.default_rng(0)
    n = 5000
    coords = rng.uniform(-0.95, 0.95, (1, n, 3)).astype(np.float32)
    rp = (0.02 * rng.standard_normal((1, n, 3))).astype(np.float32)
    rg = (0.02 * rng.standard_normal((1, n, 3))).astype(np.float32)
    print(kernel(rp, rg, coords))


# revision 28
# speedup vs baseline: 1.6395x; 1.1464x over previous
"""Trilinear scatter-add (splat) + Huber loss kernel for Trainium2, 8 NeuronCores.

Strategy
--------
reference computes:  huber_sum(splat(coords+pred) - splat(coords+gt)) over a
128^3 grid with trilinear weights and vals=1.

Key identity: the trilinear corner weights of a point with pixel coordinate p
along one axis are exactly  hat_j(p) = relu(1 - |p - j|)  for bin j in [0,128):
two adjacent nonzeros (1-frac, frac), and out-of-range corners drop out
automatically (matching grid_sample's zeros padding).

So for a batch of K points, the (y,x)-plane contribution at a fixed z-plane is
a dense matmul:   plane[y,x] += sum_k  wz_k * hat(y_k - y) * hat(x_k - x)
                             = (Wz.hatY)^T @ hatX

Sharding: the host (inside kernel(), as the sharding step) bins points by
(z0, y-half) into 258 subgroups and assigns core c the z-planes [16c, 16c+15].
Core c processes slabs s = 2*gi + yh for local groups gi = 0..16 (global z0 =
16c-1+gi): slab contributes 64 y-bins of plane z0 (weight 1-fz) and of plane
z0+1 (weight fz).  y0==63 straddlers are duplicated into the upper half.
Subgroups are padded to a fixed size so one SPMD program fits all cores;
padded records use far-away coordinates so every hat weight is exactly 0.

On device, per tile of 128 points (points live in partitions; record gives
per-partition scalar columns y, x, 1-fz, fz), TWO runtime-registered custom
DVE instructions build all weight rows (ABSOLUTE_DIFF fuses |Src0-s0|):
  HAT_PAIR_ANT:  ab[j] = relu(1-|iota2-y|) * (j<64 ? (1-fz) : 1)
                 -> [A | A+B total] for the two 64-wide y-half columns
  HAT_SCALE_ANT: xt = relu(1-|iota-x|)          (s1=1, 128-wide)
  PE :  pair[x, 0:128] += xt^T @ ab   (PSUM f32 accumulate)

Plane p, y-half yh = pairs[2p+yh][:, :64]            (A of group p)
                   + pairs[2(p-1)+yh][:, 64:]        (total of group p-1)
                   - pairs[2(p-1)+yh][:, :64]        (- A = B of group p-1);
Huber uses the branch-free identity  huber(d) = m*(|d| - m/2),  m = min(|d|, 1).
Per-core output is a [128] vector of partial sums; host adds them up.
"""

import os
import sys
import numpy as np

sys.path.insert(0, "/opt/trn_rl_repo")

from contextlib import ExitStack

import concourse.bass as bass
import concourse.tile as tile
from concourse import bacc, mybir
from concourse import dve_ops as _dvo
from concourse.bass_utils import run_bass_kernel_spmd
from concourse.dve_table_gen import dve_ver_for
from concourse.dve_spec import (
    C0, C1, C2, AluOp as _DveAluOp, Bin as _DveBin, Idx, One, Spec, Src0,
    lower as _dve_lower, maxx, relu, select as _dve_select)
from concourse.dve_uop import DveOpSpec

F32 = mybir.dt.float32
BF16 = mybir.dt.bfloat16

D = H = W = 128
N_CORES = 8
NG = 17            # groups per core: z0 in [16c-1, 16c+15]
NT_MIN = 8         # tiles of 128 points per group; actual NT sized from data

NREC = 4
XT_MOD, XT_ACT = 2, 1   # xt runs on ACT for (XT_ACT/XT_MOD) of tiles
_PAD_REC = np.array([40960.0, 40960.0, 0.0, -40960.0], dtype=np.float32)


def _hat_ref(in0, in1, s0, s1, imm2):
    ad = np.abs(in0.astype(np.float32) - s0)
    return (np.maximum(1.0 - ad, 0.0) * s1).astype(np.float32)


def _hat_pair_ref(in0, in1, s0, s1, imm2):
    flat = in0.astype(np.float32).reshape(in0.shape[0], -1)
    hat = np.maximum(1.0 - np.abs(flat - s0), 0.0)
    idx = np.arange(flat.shape[1], dtype=np.float32)[None, :]
    sc = np.where(idx < imm2, s1, np.float32(1.0))
    return (hat * sc).astype(np.float32).reshape(in0.shape)


def _register_dve_op(name, spec):
    for op in _dvo.OPS:
        if op.name == name:
            return op
    _dvo._SUB_OPCODE_FOR_NAME[name] = max(_dvo._SUB_OPCODE_FOR_NAME.values()) + 1
    assert _dvo._SUB_OPCODE_FOR_NAME[name] < 0x20
    ver = dve_ver_for("TRN2")
    tmp = DveOpSpec(name=name, opcode=_dvo._SUB_OPCODE_FOR_NAME[name],
                    uops=_dve_lower(spec, ver=ver), rd1_en=False)
    op = _dvo.DveOp(name, spec, subdim=False, uops_sha={ver: tmp.sha(ver)})
    _dvo.OPS.append(op)
    _dvo.CUSTOM_DVE_SPECS[name] = spec
    return op


def _make_hat_op():
    """out = relu(1 - |Src0 - s0|) * s1: one scaled hat row per instruction."""
    ad = _DveBin(_DveAluOp.ABSOLUTE_DIFF, Src0, C0)
    return _register_dve_op(
        "HAT_SCALE_ANT",
        Spec(body=relu(One - ad) * C1, reference=_hat_ref))


def _make_hat_pair_op():
    """out[j] = relu(1 - |Src0 - s0|) * (j < imm2 ? s1 : 1).

    One instruction against a twice-repeated iota tile yields
    [hat*(1-fz) | hat]; the unscaled half is the A+B total, so the
    B (plane z0+1) contribution is recovered at flush as total - A.
    (select(.., C1, One-C1) would need a 9th ALU stage; One is a leaf.)"""
    ad = _DveBin(_DveAluOp.ABSOLUTE_DIFF, Src0, C0)
    hat = relu(One - ad)
    sc = _dve_select(_DveBin(_DveAluOp.IS_LT, Idx, C2), C1, One)
    return _register_dve_op(
        "HAT_PAIR_ANT", Spec(body=hat * sc, reference=_hat_pair_ref))


def _pix_groups(pts: np.ndarray):
    x = ((pts[:, 0] + 1.0) * np.float32(W) - 1.0) * np.float32(0.5)
    y = ((pts[:, 1] + 1.0) * np.float32(H) - 1.0) * np.float32(0.5)
    z = ((pts[:, 2] + 1.0) * np.float32(D) - 1.0) * np.float32(0.5)
    z0 = np.floor(z).astype(np.int32)
    keep = (z0 >= -1) & (z0 <= 127)
    return x[keep], y[keep], z[keep], z0[keep]


def _subgroup_recs(pts: np.ndarray):
    """-> (recs [M,4], key [M]) with key = (z0+1)*2 + yhalf in [0, 258).
    Records store y LOCAL to the 64-bin half; y0==63 straddlers are
    duplicated into the upper half (their hat picks up only bin 64 there)."""
    x, y, z, z0 = _pix_groups(pts)
    fz = z - z0.astype(np.float32)
    y0 = np.floor(y).astype(np.int32)
    yh = (y0 >= 64).astype(np.int32)
    yl = y - 64.0 * yh.astype(np.float32)
    # [y_local, x, 1-fz, -x]; -x feeds the ACT Abs bias for the balanced xt
    recs = np.stack([yl, x, 1.0 - fz, -x], axis=1).astype(np.float32)
    key = (z0 + 1) * 2 + yh
    dup = y0 == 63
    recs_d = np.stack([y[dup] - 64.0, x[dup], 1.0 - fz[dup], -x[dup]],
                      axis=1).astype(np.float32)
    key_d = (z0[dup] + 1) * 2 + 1
    return (np.concatenate([recs, recs_d]),
            np.concatenate([key, key_d]).astype(np.int64))


def _route_points(pts: np.ndarray, nt: int):
    """pts [N,3] float32 -> per-core [2*NG, 128, nt*NREC] record arrays."""
    g_pad = nt * 128
    recs, key = _subgroup_recs(pts)

    order = np.argsort(key, kind="stable")
    keys = key[order]
    recs_s = recs[order]
    counts = np.bincount(keys, minlength=258)
    if counts.max() > g_pad:
        raise RuntimeError(f"group overflow: {counts.max()} > {g_pad}")
    starts = np.concatenate([[0], np.cumsum(counts)])

    glob = np.empty((258, g_pad, NREC), dtype=np.float32)
    glob[:] = _PAD_REC
    for g in range(258):
        n = counts[g]
        if n:
            glob[g, :n] = recs_s[starts[g]:starts[g] + n]

    ns = 2 * NG
    per_core = []
    for c in range(N_CORES):
        arr = glob[32 * c: 32 * c + ns]                       # [2NG, g_pad, NREC]
        arr = arr.reshape(ns, nt, 128, NREC).transpose(0, 2, 1, 3)
        per_core.append(np.ascontiguousarray(arr.reshape(ns, 128, nt * NREC)))
    return per_core


def build_bass(ng, nt):
    nc = bacc.Bacc(
        "TRN2", target_bir_lowering=False, debug=False, num_devices=N_CORES)
    recs_p = nc.declare_dram_parameter("recs_pred", [ng, 128, nt * NREC], F32, isOutput=False)
    recs_g = nc.declare_dram_parameter("recs_gt", [ng, 128, nt * NREC], F32, isOutput=False)
    out_part = nc.declare_dram_parameter("partials", [128, 1], F32, isOutput=True)

    hat_op = _make_hat_op()
    hat_pair_op = _make_hat_pair_op()
    iota_np = np.tile(np.arange(128, dtype=np.float32), (128, 1))
    iota_dram = nc.inline_tensor(iota_np.astype(np.float32), "iota_const")
    iota2_np = np.tile(np.concatenate([np.arange(64, dtype=np.float32)] * 2),
                       (128, 1))
    iota2_dram = nc.inline_tensor(iota2_np.astype(np.float32), "iota2_const")

    recs_in = {0: recs_p, 1: recs_g}

    with tile.TileContext(nc) as tc, ExitStack() as ctx:
        const_pool = ctx.enter_context(tc.tile_pool(name="const", bufs=1))
        rec_pool = ctx.enter_context(tc.tile_pool(name="recs", bufs=4))
        work_pool = ctx.enter_context(tc.tile_pool(name="work", bufs=4))
        ab_pool = ctx.enter_context(tc.tile_pool(name="ab", bufs=4))
        x_pool = ctx.enter_context(tc.tile_pool(name="xt", bufs=4))
        flush_pool = ctx.enter_context(tc.tile_pool(name="flush", bufs=2))
        acc_pool = ctx.enter_context(tc.tile_pool(name="acc", bufs=1))
        psum_pools = {
            0: ctx.enter_context(tc.tile_pool(name="psum_p", bufs=4, space="PSUM")),
            1: ctx.enter_context(tc.tile_pool(name="psum_g", bufs=4, space="PSUM")),
        }

        iota_sb = const_pool.tile([128, 128], F32)
        nc.sync.dma_start(iota_sb[:], iota_dram[:])
        iota2_sb = const_pool.tile([128, 128], F32, tag="iota2")
        nc.sync.dma_start(iota2_sb[:], iota2_dram[:])

        acc = acc_pool.tile([128, 128], F32)
        nc.vector.memset(acc[:], 0.0)

        pairs = {0: {}, 1: {}}  # grid -> local group idx -> psum pair tile

        for s in range(ng):            # slab s = 2*gi + yhalf
            for grid in (0, 1):
                rec = rec_pool.tile([128, nt * NREC], F32, tag="rec")
                nc.sync.dma_start(rec[:], recs_in[grid][s])

                pair = psum_pools[grid].tile([128, 128], F32, tag="pair")
                pairs[grid][s] = pair

                for t in range(nt):
                    y_col = rec[:, NREC * t + 0: NREC * t + 1]
                    x_col = rec[:, NREC * t + 1: NREC * t + 2]
                    fa_col = rec[:, NREC * t + 2: NREC * t + 3]
                    nx_col = rec[:, NREC * t + 3: NREC * t + 4]

                    ab = ab_pool.tile([128, 128], BF16, tag="ab")
                    nc.vector._custom_dve(
                        hat_pair_op, out=ab[:], in0=iota2_sb[:],
                        s0=y_col, s1=fa_col, imm2=64.0)
                    xt = x_pool.tile([128, 128], BF16, tag="xt")
                    if t % XT_MOD < XT_ACT:
                        # ACT path: |iota - x| then relu(1 - absd); keeps the
                        # otherwise-idle ACT engine carrying ~4/7 of the xt work
                        absd = work_pool.tile([128, 128], F32, tag="absd")
                        nc.scalar.activation(
                            absd[:], iota_sb[:],
                            mybir.ActivationFunctionType.Abs, bias=nx_col)
                        nc.scalar.activation(
                            xt[:], absd[:],
                            mybir.ActivationFunctionType.Relu,
                            bias=1.0, scale=-1.0)
                    else:
                        nc.vector._custom_dve(
                            hat_op, out=xt[:], in0=iota_sb[:], s0=x_col, s1=1.0)

                    nc.tensor.matmul(
                        pair[:], xt[:], ab[:],
                        start=(t == 0), stop=(t == nt - 1))

            # flush local plane lp once slabs 2lp, 2lp+1 (A) and 2lp-2, 2lp-1
            # (B) are complete, i.e. at the end of odd slab s = 2lp+1, lp >= 1
            if s % 2 == 1 and s >= 3:
                lp = (s - 1) // 2
                d = flush_pool.tile([128, 128], F32, tag="d")
                for yh in (0, 1):
                    sA = 2 * lp + yh
                    sB = 2 * (lp - 1) + yh
                    # plane half = A(sA) + (total(sB) - A(sB))
                    cP = flush_pool.tile([128, 64], F32, tag="cP")
                    nc.scalar.copy(cP[:], pairs[0][sA][:, 0:64])
                    tP = flush_pool.tile([128, 64], F32, tag="tP")
                    nc.vector.tensor_tensor(
                        tP[:], cP[:], pairs[0][sB][:, 64:128], mybir.AluOpType.add)
                    tP2 = flush_pool.tile([128, 64], F32, tag="tP2")
                    nc.vector.tensor_tensor(
                        tP2[:], tP[:], pairs[0][sB][:, 0:64], mybir.AluOpType.subtract)
                    cG = flush_pool.tile([128, 64], F32, tag="cG")
                    nc.scalar.copy(cG[:], pairs[1][sA][:, 0:64])
                    tG = flush_pool.tile([128, 64], F32, tag="tG")
                    nc.vector.tensor_tensor(
                        tG[:], cG[:], pairs[1][sB][:, 64:128], mybir.AluOpType.add)
                    tG2 = flush_pool.tile([128, 64], F32, tag="tG2")
                    nc.vector.tensor_tensor(
                        tG2[:], tG[:], pairs[1][sB][:, 0:64], mybir.AluOpType.subtract)
                    nc.vector.tensor_tensor(
                        d[:, 64 * yh: 64 * yh + 64], tP2[:], tG2[:],
                        mybir.AluOpType.subtract)
                nd = flush_pool.tile([128, 128], F32, tag="nd")
                nc.vector.tensor_scalar(
                    nd[:], d[:], -1.0, None, mybir.AluOpType.mult)
                a = flush_pool.tile([128, 128], F32, tag="a")
                nc.vector.tensor_tensor(a[:], d[:], nd[:], mybir.AluOpType.max)
                m = flush_pool.tile([128, 128], F32, tag="m")
                nc.vector.tensor_scalar(
                    m[:], a[:], 1.0, None, mybir.AluOpType.min)
                mh = flush_pool.tile([128, 128], F32, tag="mh")
                nc.vector.tensor_scalar(
                    mh[:], m[:], 0.5, None, mybir.AluOpType.mult)
                sdiff = flush_pool.tile([128, 128], F32, tag="sdiff")
                nc.vector.tensor_tensor(sdiff[:], a[:], mh[:], mybir.AluOpType.subtract)
                h = flush_pool.tile([128, 128], F32, tag="h")
                nc.vector.tensor_tensor(h[:], m[:], sdiff[:], mybir.AluOpType.mult)
                nc.vector.tensor_tensor(acc[:], acc[:], h[:], mybir.AluOpType.add)

        red = acc_pool.tile([128, 1], F32)
        nc.vector.tensor_reduce(red[:], acc[:], mybir.AxisListType.X, mybir.AluOpType.add)
        nc.sync.dma_start(out_part[:], red[:])

    nc.compile()
    return nc


_NC_CACHE = {}


def kernel(registration_pred, registration_gt, coords):
    coords = np.asarray(coords, dtype=np.float32)
    registration_pred = np.asarray(registration_pred, dtype=np.float32)
    registration_gt = np.asarray(registration_gt, dtype=np.float32)

    pred_pts = (coords + registration_pred).reshape(-1, 3).astype(np.float32)
    gt_pts = (coords + registration_gt).reshape(-1, 3).astype(np.float32)

    nt = NT_MIN
    for pts in (pred_pts, gt_pts):
        key = _subgroup_recs(pts)[1]
        nt = max(nt, int(-(-np.bincount(key, minlength=258).max() // 128)))

    pred_cores = _route_points(pred_pts, nt)
    gt_cores = _route_points(gt_pts, nt)

    key = (2 * NG, nt)
    if key not in _NC_CACHE:
        _NC_CACHE[key] = build_bass(2 * NG, nt)
    nc = _NC_CACHE[key]

    in_maps = [
        {"recs_pred": pred_cores[c], "recs_gt": gt_cores[c]}
        for c in range(N_CORES)
    ]
    res = run_bass_kernel_spmd(nc, in_maps, list(range(N_CORES)))
    total = np.float64(0.0)
    for c in range(N_CORES):
        total += np.asarray(res.results[c]["partials"], dtype=np.float64).sum()
    return np.float32(total)


if __name__ == "__main__":
    rng = np.random.default_rng(0)
    n = 5000
    coords = rng.uniform(-0.95, 0.95, (1, n, 3)).astype(np.float32)
    rp = (0.02 * rng.standard_normal((1, n, 3))).astype(np.float32)
    rg = (0.02 * rng.standard_normal((1, n, 3))).astype(np.float32)
    print(kernel(rp, rg, coords))
